# revision 40
# baseline (speedup 1.0000x reference)
"""AfmoeSparseMoeBlock on 8 Trainium2 NeuronCores.

Strategy (expert-parallel, per sharding hint):
  - Router (tiny: [T,H]@[H,16] + sigmoid + top-2) runs on host as part of
    the dispatch/shard step; it determines which token rows are shipped to
    which core.
  - Experts are sharded 2-per-core. Each core receives its two experts'
    weights plus the (transposed, bf16) gathered token batches, computes
    gate_up -> SwiGLU -> down, and scales rows by the routing weights.
  - The shared-expert MLP is tensor-parallel on its intermediate dim F:
    each core computes a 128-wide slice of the SwiGLU intermediate and a
    partial down-projection; host sums the 8 partials.
  - Combine: host scatter-adds the (already weighted) expert outputs.

Device compute is bf16 (fp32 accumulation in PSUM); outputs are fp32.
"""

import sys
from contextlib import ExitStack

sys.path.insert(0, "/opt/trn_rl_repo")

import numpy as np
import ml_dtypes

import concourse.bass as bass
import concourse.tile as tile
from concourse import bacc, mybir
from concourse.bass_utils import run_bass_kernel_spmd

# Problem constants (nn_AfmoeSparseMoeBlock_38422777430201)
B, S, H = 1, 1024, 2048
F = 1024          # moe_intermediate_size
E = 16            # num_experts
TOPK = 2
ROUTE_SCALE = 1.0
NCORES = 8
EPC = E // NCORES           # experts per core = 2
FS = F // NCORES            # shared-expert intermediate shard = 128
T = B * S

P = 128
KH = H // P      # 16 k-tiles over H
KF = F // P      # 8 k-tiles over F
NT = 512         # matmul moving free-dim / PSUM bank width (fp32)

BF = mybir.dt.bfloat16
F32 = mybir.dt.float32
bf16 = ml_dtypes.bfloat16


def _ceil_div(a, b):
    return (a + b - 1) // b


def _build_bass(cap: int) -> bass.Bass:
    """One SPMD program: shared-expert TP shard + 2 routed experts.

    All DRAM inputs are host-pre-tiled to [128, k, f] (partition-major) so
    each DMA descriptor covers a long contiguous per-partition segment --
    descriptor generation on the DGE sequencers is the scarce resource.
    """
    nc = bacc.Bacc("TRN2", target_bir_lowering=False, debug=False,
                   num_devices=NCORES)

    xT = nc.declare_dram_parameter("xT", [P, KH, T], BF, isOutput=False)
    sg = nc.declare_dram_parameter("sg", [P, KH, FS], BF, isOutput=False)
    su = nc.declare_dram_parameter("su", [P, KH, FS], BF, isOutput=False)
    sd = nc.declare_dram_parameter("sd", [FS, H], BF, isOutput=False)
    xe = [nc.declare_dram_parameter(f"xe{s}", [P, KH, cap], BF, isOutput=False)
          for s in range(EPC)]
    wg = [nc.declare_dram_parameter(f"wg{s}", [P, KH, F], BF, isOutput=False)
          for s in range(EPC)]
    wu = [nc.declare_dram_parameter(f"wu{s}", [P, KH, F], BF, isOutput=False)
          for s in range(EPC)]
    wd = [nc.declare_dram_parameter(f"wd{s}", [P, KF, H], BF, isOutput=False)
          for s in range(EPC)]
    wt = [nc.declare_dram_parameter(f"wt{s}", [cap, 1], F32, isOutput=False)
          for s in range(EPC)]
    shp = nc.declare_dram_parameter("shared_part", [T, H], BF, isOutput=True)
    ys = [nc.declare_dram_parameter(f"y{s}", [cap, H], BF, isOutput=True)
          for s in range(EPC)]

    ncp = _ceil_div(cap, P)       # token tiles per expert batch
    nsl = _ceil_div(cap, NT)      # moving-dim slices per expert batch

    def nslice(n):
        lo = n * NT
        return lo, min(NT, cap - lo)

    with tile.TileContext(nc) as tc, ExitStack() as ctx:
        const = ctx.enter_context(tc.tile_pool(name="resident", bufs=1))
        psum = ctx.enter_context(tc.tile_pool(name="psum", bufs=8, space="PSUM"))
        work = ctx.enter_context(tc.tile_pool(name="work", bufs=3))
        wpool = ctx.enter_context(tc.tile_pool(name="wstream", bufs=6))
        epool = ctx.enter_context(tc.tile_pool(name="eact", bufs=2))

        # ---- resident loads -------------------------------------------------
        # Everything loads through the sync queue so FIFO order holds:
        # small activation inputs FIRST, then the bulk weight streams (the
        # 16 DMA engines drain descriptors in issue order). Stores go on
        # gpsimd.
        xT_sb = const.tile([P, KH, T], BF)
        sg_sb = const.tile([P, KH, FS], BF)
        su_sb = const.tile([P, KH, FS], BF)
        sd_sb = const.tile([P, H], BF)
        xe_sb = [const.tile([P, KH, cap], BF, tag=f"xe_sb{s}", name=f"xe_sb{s}")
                 for s in range(EPC)]
        wt_sb = [const.tile([P, ncp, 1], F32, tag=f"wt_sb{s}", name=f"wt_sb{s}")
                 for s in range(EPC)]
        nc.sync.dma_start(xT_sb[:, 0:2, :], xT[:, 0:2, :])
        nc.sync.dma_start(sg_sb[:, 0:8, :], sg[:, 0:8, :])
        nc.sync.dma_start(su_sb[:, 0:8, :], su[:, 0:8, :])
        nc.sync.dma_start(xe_sb[0][:], xe[0][:])
        nc.sync.dma_start(xT_sb[:, 2:4, :], xT[:, 2:4, :])
        nc.sync.dma_start(xT_sb[:, 4:8, :], xT[:, 4:8, :])
        nc.sync.dma_start(sg_sb[:, 8:16, :], sg[:, 8:16, :])
        nc.sync.dma_start(su_sb[:, 8:16, :], su[:, 8:16, :])
        nc.sync.dma_start(xT_sb[:, 8:16, :], xT[:, 8:16, :])
        nc.sync.dma_start(xe_sb[1][:], xe[1][:])
        nc.sync.dma_start(sd_sb[:], sd[:, :])
        for s in range(EPC):
            for c in range(ncp):
                mc = min(P, cap - c * P)
                nc.sync.dma_start(wt_sb[s][:mc, c, :],
                                  wt[s][c * P:c * P + mc, :])

        # ---- shared expert gate/up (TP shard on F) --------------------------
        # act_s[FS, T] = silu(sg_shard.T @ x.T) * (su_shard.T @ x.T)
        act_s = const.tile([P, T], BF)
        for tsl in range(T // NT):
            pg = psum.tile([P, NT], F32, tag="mm", name=f"pg{tsl}")
            pu = psum.tile([P, NT], F32, tag="mm", name=f"pu{tsl}")
            for k in range(KH):
                nc.tensor.matmul(pg[:], sg_sb[:, k, :],
                                 xT_sb[:, k, tsl * NT:(tsl + 1) * NT],
                                 start=(k == 0), stop=(k == KH - 1))
            for k in range(KH):
                nc.tensor.matmul(pu[:], su_sb[:, k, :],
                                 xT_sb[:, k, tsl * NT:(tsl + 1) * NT],
                                 start=(k == 0), stop=(k == KH - 1))
            tmp = work.tile([P, NT], F32, tag="silu_s", name=f"tmp{tsl}")
            nc.scalar.activation(tmp[:], pg[:],
                                 mybir.ActivationFunctionType.Silu)
            nc.vector.tensor_mul(act_s[:, tsl * NT:(tsl + 1) * NT], tmp[:],
                                 pu[:])

        def shared_down(t0, t1):
            # partial down-proj: shared_part[T, H] = act_s.T @ sd_shard
            for t in range(t0, t1):
                ob = work.tile([P, H], BF, tag="out_s", name=f"ob{t}")
                for h in range(H // NT):
                    po = psum.tile([P, NT], F32, tag="mm", name=f"po{t}_{h}")
                    nc.tensor.matmul(po[:], act_s[:, t * P:(t + 1) * P],
                                     sd_sb[:, h * NT:(h + 1) * NT],
                                     start=True, stop=True)
                    # alternate DVE/ACT so neither FIFO gates the gu/down
                    # boundary evictions
                    if (t * 4 + h) % 2 == 0:
                        nc.vector.tensor_copy(ob[:, h * NT:(h + 1) * NT], po[:])
                    else:
                        nc.scalar.activation(ob[:, h * NT:(h + 1) * NT], po[:],
                                             mybir.ActivationFunctionType.Copy)
                nc.gpsimd.dma_start(shp[t * P:(t + 1) * P, :], ob[:])

        def expert_gu_half(s, half, silu_g, act_e):
            # one half of guT[2F, cap] over H: 8 accumulation groups, one
            # PSUM bank per group (packing groups into a bank is a HW fault)
            wsrc = wg[s] if half == 0 else wu[s]
            assert cap <= NT, "expert batch over 512 tokens unsupported"
            ps = [psum.tile([P, cap], F32, tag="mm", name=f"ps_{s}_{half}_{f}")
                  for f in range(KF)]
            for k0 in range(0, KH, 4):  # 1MB 4-k-tile chunks
                wchunk = wpool.tile([P, 4, F], BF, tag="wgu_chunk",
                                    name=f"wgu_{s}_{half}_{k0}")
                nc.sync.dma_start(wchunk[:], wsrc[:, k0:k0 + 4, :])
                for dk in range(4):
                    k = k0 + dk
                    for f in range(KF):
                        nc.tensor.matmul(
                            ps[f][:],
                            wchunk[:, dk, f * P:(f + 1) * P],
                            xe_sb[s][:, k, :],
                            start=(k == 0), stop=(k == KH - 1))
            for f in range(KF):
                if half == 0:  # gate -> silu
                    nc.scalar.activation(silu_g[:, f, :], ps[f][:],
                                         mybir.ActivationFunctionType.Silu)
                else:          # up -> silu(g) * u
                    nc.vector.tensor_mul(act_e[:, f, :],
                                         silu_g[:, f, :], ps[f][:])

        def expert_down(s, act_e):
            # down-proj: y[cap, H] = (act_e.T @ wd) * wt, (c, h) psum groups
            pairs = [(c, h) for c in range(ncp) for h in range(H // NT)]
            for g0 in range(0, len(pairs), 8):
                grp = pairs[g0:g0 + 8]
                pY = {ch: psum.tile([P, NT], F32, tag="mm",
                                    name=f"pY_{s}_{ch[0]}_{ch[1]}")
                      for ch in grp}
                for fk0 in range(0, KF, 2):  # 2 f-strips per DMA (1MB)
                    wdstrip = wpool.tile([P, 2, H], BF, tag="wd_strip",
                                         name=f"wd_{s}_{fk0}")
                    nc.sync.dma_start(wdstrip[:], wd[s][:, fk0:fk0 + 2, :])
                    for dk in range(2):
                        fk = fk0 + dk
                        for (c, h) in grp:
                            mc = min(P, cap - c * P)
                            nc.tensor.matmul(
                                pY[(c, h)][:mc, :],
                                act_e[:, fk, c * P:c * P + mc],
                                wdstrip[:, dk, h * NT:(h + 1) * NT],
                                start=(fk == 0), stop=(fk == KF - 1))
                ycs = {}
                for (c, h) in grp:
                    mc = min(P, cap - c * P)
                    if c not in ycs:
                        ycs[c] = work.tile([P, H], BF, tag="yout",
                                           name=f"yb_{s}_{c}_{h}")
                    dst = ycs[c][:mc, h * NT:(h + 1) * NT]
                    if h % 2 == 0:  # split evictions across DVE and ACT
                        nc.vector.tensor_scalar_mul(dst, pY[(c, h)][:mc, :],
                                                    wt_sb[s][:mc, c, :])
                    else:
                        nc.scalar.activation(dst, pY[(c, h)][:mc, :],
                                             mybir.ActivationFunctionType.Copy,
                                             scale=wt_sb[s][:mc, c, :])
                for c, yb in ycs.items():
                    mc = min(P, cap - c * P)
                    nc.gpsimd.dma_start(ys[s][c * P:c * P + mc, :], yb[:mc, :])

        # order: shared-down after down(0), where its serialized
        # MM->evict chain hides under the DMA-paced expert weight streams
        eact = [(epool.tile([P, KF, cap], F32, tag="silu_g", name=f"silu_g{s}"),
                 epool.tile([P, KF, cap], BF, tag="act_e", name=f"act_e{s}"))
                for s in range(EPC)]
        shared_down(0, 8)
        expert_gu_half(0, 0, *eact[0])
        expert_gu_half(0, 1, *eact[0])
        expert_down(0, eact[0][1])
        expert_gu_half(1, 0, *eact[1])
        expert_gu_half(1, 1, *eact[1])
        expert_down(1, eact[1][1])

    nc.compile()
    return nc


def _route_host(x, gate_w, expert_bias):
    """Replicates the reference router in fp32 numpy."""
    logits = x @ gate_w                                    # [T, E]
    scores = 1.0 / (1.0 + np.exp(-logits, dtype=np.float32))
    sel = np.argsort(-(scores + expert_bias[None, :]), axis=1, kind="stable")[:, :TOPK]
    top = np.take_along_axis(scores, sel, axis=1)          # [T, K]
    top = top / (top.sum(-1, keepdims=True) + 1e-20)
    top = top * ROUTE_SCALE
    return sel, top.astype(np.float32)


def _ensure_ntff_hook():
    """The image's antenv lacks axon_hooks; inject it and register the
    NTFF profile hook so trace=True yields exec_time_ns."""
    import types
    try:
        from antenv import axon_hooks  # noqa: F401
        return
    except ImportError:
        pass
    try:
        import antenv
        from trn_agent_boot.trn_boot import _ntff_profile_via_ctypes
        mod = types.ModuleType("antenv.axon_hooks")
        mod._hook = None

        def _set(h):
            mod._hook = h

        def _get():
            return mod._hook

        mod.set_axon_ntff_profile_hook = _set
        mod.get_axon_ntff_profile_hook = _get
        sys.modules["antenv.axon_hooks"] = mod
        antenv.axon_hooks = mod
        _set(_ntff_profile_via_ctypes("/opt/axon/libaxon_pjrt.so"))
    except Exception as e:  # tracing degrades, run still works
        print(f"ntff hook setup failed: {e}")


def _run(inputs, trace=False, trace_cores=None):
    if trace:
        _ensure_ntff_hook()
    x = np.asarray(inputs["hidden_states"], np.float32).reshape(T, H)
    gate_w = np.asarray(inputs["gate_w"], np.float32)
    expert_bias = np.asarray(inputs["expert_bias"], np.float32)
    sgw = np.asarray(inputs["shared_gate_w"], np.float32)
    suw = np.asarray(inputs["shared_up_w"], np.float32)
    sdw = np.asarray(inputs["shared_down_w"], np.float32)
    egu = np.asarray(inputs["experts_gate_up"], np.float32)
    edn = np.asarray(inputs["experts_down"], np.float32)

    # --- host router + dispatch (the shard step) ---
    sel, top = _route_host(x, gate_w, expert_bias)
    idx = [np.where(sel == e)[0] for e in range(E)]        # token ids per expert
    wts = [top[sel == e] for e in range(E)]                # routing weight per token
    cap = max(8, -(-max(len(i) for i in idx) // 16) * 16)  # pad to mult of 16
    if cap > T:
        cap = T

    def ptile(a):
        """[K*128, f] row-major -> [128, K, f] partition-major bf16."""
        k = a.shape[0] // P
        return np.ascontiguousarray(
            a.reshape(k, P, a.shape[1]).transpose(1, 0, 2)).astype(bf16)

    xT_bf = ptile(x.T)                                     # [P, KH, T]

    def gathered(e):
        xt = np.zeros((H, cap), np.float32)
        n = len(idx[e])
        xt[:, :n] = x[idx[e]].T
        w = np.zeros((cap, 1), np.float32)
        w[:n, 0] = wts[e]
        return ptile(xt), w

    in_maps = []
    for core in range(NCORES):
        m = {
            "xT": xT_bf,
            "sg": ptile(sgw[:, core * FS:(core + 1) * FS]),
            "su": ptile(suw[:, core * FS:(core + 1) * FS]),
            "sd": np.ascontiguousarray(sdw[core * FS:(core + 1) * FS, :]).astype(bf16),
        }
        for s in range(EPC):
            e = core * EPC + s
            xt, w = gathered(e)
            m[f"xe{s}"] = xt
            m[f"wt{s}"] = w
            m[f"wg{s}"] = ptile(egu[e][:, :F])
            m[f"wu{s}"] = ptile(egu[e][:, F:])
            m[f"wd{s}"] = ptile(edn[e])
        in_maps.append(m)

    nc = _build_bass(cap)
    res = run_bass_kernel_spmd(nc, in_maps, list(range(NCORES)),
                               trace=trace, trace_cores=trace_cores)

    # --- host combine (unshard) ---
    out = np.zeros((T, H), np.float32)
    for core in range(NCORES):
        out += res.results[core]["shared_part"].astype(np.float32)
        for s in range(EPC):
            e = core * EPC + s
            n = len(idx[e])
            if n:  # token ids within one expert are unique -> plain fancy add
                out[idx[e]] += res.results[core][f"y{s}"][:n].astype(np.float32)
    return out.reshape(B, S, H), res


def kernel(**inputs) -> np.ndarray:
    out, _ = _run(inputs)
    return out


# revision 41
# speedup vs baseline: 1.0314x; 1.0314x over previous
"""AfmoeSparseMoeBlock on 8 Trainium2 NeuronCores.

Strategy (expert-parallel, per sharding hint):
  - Router (tiny: [T,H]@[H,16] + sigmoid + top-2) runs on host as part of
    the dispatch/shard step; it determines which token rows are shipped to
    which core.
  - Experts are sharded 2-per-core. Each core receives its two experts'
    weights plus the (transposed, bf16) gathered token batches, computes
    gate_up -> SwiGLU -> down, and scales rows by the routing weights.
  - The shared-expert MLP is tensor-parallel on its intermediate dim F:
    each core computes a 128-wide slice of the SwiGLU intermediate and a
    partial down-projection; host sums the 8 partials.
  - Combine: host scatter-adds the (already weighted) expert outputs.

Device compute is bf16 (fp32 accumulation in PSUM); outputs are fp32.
"""

import sys
from contextlib import ExitStack

sys.path.insert(0, "/opt/trn_rl_repo")

import numpy as np
import ml_dtypes

import concourse.bass as bass
import concourse.tile as tile
from concourse import bacc, mybir
from concourse.bass_utils import run_bass_kernel_spmd

# Problem constants (nn_AfmoeSparseMoeBlock_38422777430201)
B, S, H = 1, 1024, 2048
F = 1024          # moe_intermediate_size
E = 16            # num_experts
TOPK = 2
ROUTE_SCALE = 1.0
NCORES = 8
EPC = E // NCORES           # experts per core = 2
FS = F // NCORES            # shared-expert intermediate shard = 128
T = B * S

P = 128
KH = H // P      # 16 k-tiles over H
KF = F // P      # 8 k-tiles over F
NT = 512         # matmul moving free-dim / PSUM bank width (fp32)

BF = mybir.dt.bfloat16
F32 = mybir.dt.float32
bf16 = ml_dtypes.bfloat16


def _ceil_div(a, b):
    return (a + b - 1) // b


def _build_bass(cap: int) -> bass.Bass:
    """One SPMD program: shared-expert TP shard + 2 routed experts.

    All DRAM inputs are host-pre-tiled to [128, k, f] (partition-major) so
    each DMA descriptor covers a long contiguous per-partition segment --
    descriptor generation on the DGE sequencers is the scarce resource.
    """
    nc = bacc.Bacc("TRN2", target_bir_lowering=False, debug=False,
                   num_devices=NCORES)

    xT = nc.declare_dram_parameter("xT", [P, KH, T], BF, isOutput=False)
    sg = nc.declare_dram_parameter("sg", [P, KH, FS], BF, isOutput=False)
    su = nc.declare_dram_parameter("su", [P, KH, FS], BF, isOutput=False)
    sd = nc.declare_dram_parameter("sd", [FS, H], BF, isOutput=False)
    xe = [nc.declare_dram_parameter(f"xe{s}", [P, KH, cap], BF, isOutput=False)
          for s in range(EPC)]
    wg = [nc.declare_dram_parameter(f"wg{s}", [P, KH, F], BF, isOutput=False)
          for s in range(EPC)]
    wu = [nc.declare_dram_parameter(f"wu{s}", [P, KH, F], BF, isOutput=False)
          for s in range(EPC)]
    wd = [nc.declare_dram_parameter(f"wd{s}", [P, KF, H], BF, isOutput=False)
          for s in range(EPC)]
    wt = [nc.declare_dram_parameter(f"wt{s}", [cap, 1], F32, isOutput=False)
          for s in range(EPC)]
    shp = nc.declare_dram_parameter("shared_part", [T, H], BF, isOutput=True)
    ys = [nc.declare_dram_parameter(f"y{s}", [cap, H], BF, isOutput=True)
          for s in range(EPC)]

    ncp = _ceil_div(cap, P)       # token tiles per expert batch
    nsl = _ceil_div(cap, NT)      # moving-dim slices per expert batch

    def nslice(n):
        lo = n * NT
        return lo, min(NT, cap - lo)

    with tile.TileContext(nc) as tc, ExitStack() as ctx:
        const = ctx.enter_context(tc.tile_pool(name="resident", bufs=1))
        psum = ctx.enter_context(tc.tile_pool(name="psum", bufs=8, space="PSUM"))
        work = ctx.enter_context(tc.tile_pool(name="work", bufs=3))
        wpool = ctx.enter_context(tc.tile_pool(name="wstream", bufs=6))
        epool = ctx.enter_context(tc.tile_pool(name="eact", bufs=2))

        # ---- resident loads -------------------------------------------------
        # Everything loads through the sync queue so FIFO order holds:
        # small activation inputs FIRST, then the bulk weight streams (the
        # 16 DMA engines drain descriptors in issue order). Stores go on
        # gpsimd.
        xT_sb = const.tile([P, KH, T], BF)
        sg_sb = const.tile([P, KH, FS], BF)
        su_sb = const.tile([P, KH, FS], BF)
        sd_sb = const.tile([P, H], BF)
        xe_sb = [const.tile([P, KH, cap], BF, tag=f"xe_sb{s}", name=f"xe_sb{s}")
                 for s in range(EPC)]
        wt_sb = [const.tile([P, ncp, 1], F32, tag=f"wt_sb{s}", name=f"wt_sb{s}")
                 for s in range(EPC)]
        nc.sync.dma_start(xT_sb[:, 0:2, :], xT[:, 0:2, :])
        nc.sync.dma_start(sg_sb[:, 0:8, :], sg[:, 0:8, :])
        nc.sync.dma_start(su_sb[:, 0:8, :], su[:, 0:8, :])
        nc.sync.dma_start(xe_sb[0][:], xe[0][:])
        nc.sync.dma_start(xT_sb[:, 2:4, :], xT[:, 2:4, :])
        nc.sync.dma_start(xT_sb[:, 4:8, :], xT[:, 4:8, :])
        nc.sync.dma_start(sg_sb[:, 8:16, :], sg[:, 8:16, :])
        nc.sync.dma_start(su_sb[:, 8:16, :], su[:, 8:16, :])
        nc.sync.dma_start(xT_sb[:, 8:16, :], xT[:, 8:16, :])
        nc.sync.dma_start(xe_sb[1][:], xe[1][:])
        nc.sync.dma_start(sd_sb[:], sd[:, :])
        for s in range(EPC):
            for c in range(ncp):
                mc = min(P, cap - c * P)
                nc.sync.dma_start(wt_sb[s][:mc, c, :],
                                  wt[s][c * P:c * P + mc, :])

        # ---- shared expert gate/up (TP shard on F) --------------------------
        # act_s[FS, T] = silu(sg_shard.T @ x.T) * (su_shard.T @ x.T)
        # HAM warmup: ~4us of junk matmuls during the DMA ramp so real
        # matmuls start at 2.4 GHz. Results land in the first psum tile
        # and are discarded by the real chain's start=True.
        warm = const.tile([P, P], BF)
        nc.gpsimd.memset(warm[:], 0.0)
        pwarm = psum.tile([P, NT], F32, tag="mm", name="pwarm")
        for i in range(40):
            nc.tensor.matmul(pwarm[:, :P], warm[:], warm[:],
                             start=True, stop=True)

        act_s = const.tile([P, T], BF)
        for tsl in range(T // NT):
            pg = psum.tile([P, NT], F32, tag="mm", name=f"pg{tsl}")
            pu = psum.tile([P, NT], F32, tag="mm", name=f"pu{tsl}")
            for k in range(KH):
                nc.tensor.matmul(pg[:], sg_sb[:, k, :],
                                 xT_sb[:, k, tsl * NT:(tsl + 1) * NT],
                                 start=(k == 0), stop=(k == KH - 1))
            for k in range(KH):
                nc.tensor.matmul(pu[:], su_sb[:, k, :],
                                 xT_sb[:, k, tsl * NT:(tsl + 1) * NT],
                                 start=(k == 0), stop=(k == KH - 1))
            tmp = work.tile([P, NT], F32, tag="silu_s", name=f"tmp{tsl}")
            nc.scalar.activation(tmp[:], pg[:],
                                 mybir.ActivationFunctionType.Silu)
            nc.vector.tensor_mul(act_s[:, tsl * NT:(tsl + 1) * NT], tmp[:],
                                 pu[:])

        def shared_down(t0, t1):
            # partial down-proj: shared_part[T, H] = act_s.T @ sd_shard
            for t in range(t0, t1):
                ob = work.tile([P, H], BF, tag="out_s", name=f"ob{t}")
                for h in range(H // NT):
                    po = psum.tile([P, NT], F32, tag="mm", name=f"po{t}_{h}")
                    nc.tensor.matmul(po[:], act_s[:, t * P:(t + 1) * P],
                                     sd_sb[:, h * NT:(h + 1) * NT],
                                     start=True, stop=True)
                    # alternate DVE/ACT so neither FIFO gates the gu/down
                    # boundary evictions
                    if (t * 4 + h) % 2 == 0:
                        nc.vector.tensor_copy(ob[:, h * NT:(h + 1) * NT], po[:])
                    else:
                        nc.scalar.activation(ob[:, h * NT:(h + 1) * NT], po[:],
                                             mybir.ActivationFunctionType.Copy)
                nc.gpsimd.dma_start(shp[t * P:(t + 1) * P, :], ob[:])

        def expert_gu_half(s, half, silu_g, act_e):
            # one half of guT[2F, cap] over H: 8 accumulation groups, one
            # PSUM bank per group (packing groups into a bank is a HW fault)
            wsrc = wg[s] if half == 0 else wu[s]
            assert cap <= NT, "expert batch over 512 tokens unsupported"
            ps = [psum.tile([P, cap], F32, tag="mm", name=f"ps_{s}_{half}_{f}")
                  for f in range(KF)]
            for k0 in range(0, KH, 4):  # 1MB 4-k-tile chunks
                wchunk = wpool.tile([P, 4, F], BF, tag="wgu_chunk",
                                    name=f"wgu_{s}_{half}_{k0}")
                nc.sync.dma_start(wchunk[:], wsrc[:, k0:k0 + 4, :])
                for dk in range(4):
                    k = k0 + dk
                    for f in range(KF):
                        nc.tensor.matmul(
                            ps[f][:],
                            wchunk[:, dk, f * P:(f + 1) * P],
                            xe_sb[s][:, k, :],
                            start=(k == 0), stop=(k == KH - 1))
            for f in range(KF):
                if half == 0:  # gate -> silu
                    nc.scalar.activation(silu_g[:, f, :], ps[f][:],
                                         mybir.ActivationFunctionType.Silu)
                else:          # up -> silu(g) * u
                    nc.vector.tensor_mul(act_e[:, f, :],
                                         silu_g[:, f, :], ps[f][:])

        def expert_down(s, act_e):
            # down-proj: y[cap, H] = (act_e.T @ wd) * wt, (c, h) psum groups
            pairs = [(c, h) for c in range(ncp) for h in range(H // NT)]
            for g0 in range(0, len(pairs), 8):
                grp = pairs[g0:g0 + 8]
                pY = {ch: psum.tile([P, NT], F32, tag="mm",
                                    name=f"pY_{s}_{ch[0]}_{ch[1]}")
                      for ch in grp}
                for fk0 in range(0, KF, 2):  # 2 f-strips per DMA (1MB)
                    wdstrip = wpool.tile([P, 2, H], BF, tag="wd_strip",
                                         name=f"wd_{s}_{fk0}")
                    nc.sync.dma_start(wdstrip[:], wd[s][:, fk0:fk0 + 2, :])
                    for dk in range(2):
                        fk = fk0 + dk
                        for (c, h) in grp:
                            mc = min(P, cap - c * P)
                            nc.tensor.matmul(
                                pY[(c, h)][:mc, :],
                                act_e[:, fk, c * P:c * P + mc],
                                wdstrip[:, dk, h * NT:(h + 1) * NT],
                                start=(fk == 0), stop=(fk == KF - 1))
                ycs = {}
                for (c, h) in grp:
                    mc = min(P, cap - c * P)
                    if c not in ycs:
                        ycs[c] = work.tile([P, H], BF, tag="yout",
                                           name=f"yb_{s}_{c}_{h}")
                    dst = ycs[c][:mc, h * NT:(h + 1) * NT]
                    if h % 2 == 0:  # split evictions across DVE and ACT
                        nc.vector.tensor_scalar_mul(dst, pY[(c, h)][:mc, :],
                                                    wt_sb[s][:mc, c, :])
                    else:
                        nc.scalar.activation(dst, pY[(c, h)][:mc, :],
                                             mybir.ActivationFunctionType.Copy,
                                             scale=wt_sb[s][:mc, c, :])
                for c, yb in ycs.items():
                    mc = min(P, cap - c * P)
                    nc.gpsimd.dma_start(ys[s][c * P:c * P + mc, :], yb[:mc, :])

        # order: shared-down after down(0), where its serialized
        # MM->evict chain hides under the DMA-paced expert weight streams
        eact = [(epool.tile([P, KF, cap], F32, tag="silu_g", name=f"silu_g{s}"),
                 epool.tile([P, KF, cap], BF, tag="act_e", name=f"act_e{s}"))
                for s in range(EPC)]
        shared_down(0, 8)
        expert_gu_half(0, 0, *eact[0])
        expert_gu_half(0, 1, *eact[0])
        expert_down(0, eact[0][1])
        expert_gu_half(1, 0, *eact[1])
        expert_gu_half(1, 1, *eact[1])
        expert_down(1, eact[1][1])

    nc.compile()
    return nc


def _route_host(x, gate_w, expert_bias):
    """Replicates the reference router in fp32 numpy."""
    logits = x @ gate_w                                    # [T, E]
    scores = 1.0 / (1.0 + np.exp(-logits, dtype=np.float32))
    sel = np.argsort(-(scores + expert_bias[None, :]), axis=1, kind="stable")[:, :TOPK]
    top = np.take_along_axis(scores, sel, axis=1)          # [T, K]
    top = top / (top.sum(-1, keepdims=True) + 1e-20)
    top = top * ROUTE_SCALE
    return sel, top.astype(np.float32)


def _ensure_ntff_hook():
    """The image's antenv lacks axon_hooks; inject it and register the
    NTFF profile hook so trace=True yields exec_time_ns."""
    import types
    try:
        from antenv import axon_hooks  # noqa: F401
        return
    except ImportError:
        pass
    try:
        import antenv
        from trn_agent_boot.trn_boot import _ntff_profile_via_ctypes
        mod = types.ModuleType("antenv.axon_hooks")
        mod._hook = None

        def _set(h):
            mod._hook = h

        def _get():
            return mod._hook

        mod.set_axon_ntff_profile_hook = _set
        mod.get_axon_ntff_profile_hook = _get
        sys.modules["antenv.axon_hooks"] = mod
        antenv.axon_hooks = mod
        _set(_ntff_profile_via_ctypes("/opt/axon/libaxon_pjrt.so"))
    except Exception as e:  # tracing degrades, run still works
        print(f"ntff hook setup failed: {e}")


def _run(inputs, trace=False, trace_cores=None):
    if trace:
        _ensure_ntff_hook()
    x = np.asarray(inputs["hidden_states"], np.float32).reshape(T, H)
    gate_w = np.asarray(inputs["gate_w"], np.float32)
    expert_bias = np.asarray(inputs["expert_bias"], np.float32)
    sgw = np.asarray(inputs["shared_gate_w"], np.float32)
    suw = np.asarray(inputs["shared_up_w"], np.float32)
    sdw = np.asarray(inputs["shared_down_w"], np.float32)
    egu = np.asarray(inputs["experts_gate_up"], np.float32)
    edn = np.asarray(inputs["experts_down"], np.float32)

    # --- host router + dispatch (the shard step) ---
    sel, top = _route_host(x, gate_w, expert_bias)
    idx = [np.where(sel == e)[0] for e in range(E)]        # token ids per expert
    wts = [top[sel == e] for e in range(E)]                # routing weight per token
    cap = max(8, -(-max(len(i) for i in idx) // 16) * 16)  # pad to mult of 16
    if cap > T:
        cap = T

    def ptile(a):
        """[K*128, f] row-major -> [128, K, f] partition-major bf16."""
        k = a.shape[0] // P
        return np.ascontiguousarray(
            a.reshape(k, P, a.shape[1]).transpose(1, 0, 2)).astype(bf16)

    xT_bf = ptile(x.T)                                     # [P, KH, T]

    def gathered(e):
        xt = np.zeros((H, cap), np.float32)
        n = len(idx[e])
        xt[:, :n] = x[idx[e]].T
        w = np.zeros((cap, 1), np.float32)
        w[:n, 0] = wts[e]
        return ptile(xt), w

    in_maps = []
    for core in range(NCORES):
        m = {
            "xT": xT_bf,
            "sg": ptile(sgw[:, core * FS:(core + 1) * FS]),
            "su": ptile(suw[:, core * FS:(core + 1) * FS]),
            "sd": np.ascontiguousarray(sdw[core * FS:(core + 1) * FS, :]).astype(bf16),
        }
        for s in range(EPC):
            e = core * EPC + s
            xt, w = gathered(e)
            m[f"xe{s}"] = xt
            m[f"wt{s}"] = w
            m[f"wg{s}"] = ptile(egu[e][:, :F])
            m[f"wu{s}"] = ptile(egu[e][:, F:])
            m[f"wd{s}"] = ptile(edn[e])
        in_maps.append(m)

    nc = _build_bass(cap)
    res = run_bass_kernel_spmd(nc, in_maps, list(range(NCORES)),
                               trace=trace, trace_cores=trace_cores)

    # --- host combine (unshard) ---
    out = np.zeros((T, H), np.float32)
    for core in range(NCORES):
        out += res.results[core]["shared_part"].astype(np.float32)
        for s in range(EPC):
            e = core * EPC + s
            n = len(idx[e])
            if n:  # token ids within one expert are unique -> plain fancy add
                out[idx[e]] += res.results[core][f"y{s}"][:n].astype(np.float32)
    return out.reshape(B, S, H), res


def kernel(**inputs) -> np.ndarray:
    out, _ = _run(inputs)
    return out


# revision 50
# speedup vs baseline: 1.0681x; 1.0356x over previous
"""AfmoeSparseMoeBlock on 8 Trainium2 NeuronCores.

Strategy (expert-parallel, per sharding hint):
  - Router (tiny: [T,H]@[H,16] + sigmoid + top-2) runs on host as part of
    the dispatch/shard step; it determines which token rows are shipped to
    which core.
  - Experts are sharded 2-per-core. Each core receives its two experts'
    weights plus the (transposed, bf16) gathered token batches, computes
    gate_up -> SwiGLU -> down, and scales rows by the routing weights.
  - The shared-expert MLP is tensor-parallel on its intermediate dim F:
    each core computes a 128-wide slice of the SwiGLU intermediate and a
    partial down-projection; host sums the 8 partials.
  - Combine: host scatter-adds the (already weighted) expert outputs.

Device compute is bf16 (fp32 accumulation in PSUM); outputs are fp32.
"""

import sys
from contextlib import ExitStack

sys.path.insert(0, "/opt/trn_rl_repo")

import numpy as np
import ml_dtypes

import concourse.bass as bass
import concourse.tile as tile
from concourse import bacc, mybir
from concourse.bass_utils import run_bass_kernel_spmd

# Problem constants (nn_AfmoeSparseMoeBlock_38422777430201)
B, S, H = 1, 1024, 2048
F = 1024          # moe_intermediate_size
E = 16            # num_experts
TOPK = 2
ROUTE_SCALE = 1.0
NCORES = 8
EPC = E // NCORES           # experts per core = 2
NFS = 4                     # shared expert: 4-way F shard x 2-way token shard
NTS = NCORES // NFS
FS = F // NFS               # shared-expert intermediate shard = 256
TS = (B * S) // NTS         # shared-expert token shard = 512
T = B * S

P = 128
KH = H // P      # 16 k-tiles over H
KF = F // P      # 8 k-tiles over F
NT = 512         # matmul moving free-dim / PSUM bank width (fp32)

BF = mybir.dt.bfloat16
F32 = mybir.dt.float32
bf16 = ml_dtypes.bfloat16


def _ceil_div(a, b):
    return (a + b - 1) // b


def _build_bass(cap: int) -> bass.Bass:
    """One SPMD program: shared-expert TP shard + 2 routed experts.

    All DRAM inputs are host-pre-tiled to [128, k, f] (partition-major) so
    each DMA descriptor covers a long contiguous per-partition segment --
    descriptor generation on the DGE sequencers is the scarce resource.
    """
    nc = bacc.Bacc("TRN2", target_bir_lowering=False, debug=False,
                   num_devices=NCORES)

    xT = nc.declare_dram_parameter("xT", [P, KH, TS], BF, isOutput=False)
    sg = nc.declare_dram_parameter("sg", [P, KH, FS], BF, isOutput=False)
    su = nc.declare_dram_parameter("su", [P, KH, FS], BF, isOutput=False)
    sd = nc.declare_dram_parameter("sd", [P, FS // P, H], BF, isOutput=False)
    xe = [nc.declare_dram_parameter(f"xe{s}", [P, KH, cap], BF, isOutput=False)
          for s in range(EPC)]
    wg = [nc.declare_dram_parameter(f"wg{s}", [P, KH, F], BF, isOutput=False)
          for s in range(EPC)]
    wu = [nc.declare_dram_parameter(f"wu{s}", [P, KH, F], BF, isOutput=False)
          for s in range(EPC)]
    wd = [nc.declare_dram_parameter(f"wd{s}", [P, KF, H], BF, isOutput=False)
          for s in range(EPC)]
    wt = [nc.declare_dram_parameter(f"wt{s}", [cap, 1], F32, isOutput=False)
          for s in range(EPC)]
    shp = nc.declare_dram_parameter("shared_part", [TS, H], BF, isOutput=True)
    ys = [nc.declare_dram_parameter(f"y{s}", [cap, H], BF, isOutput=True)
          for s in range(EPC)]

    ncp = _ceil_div(cap, P)       # token tiles per expert batch
    nsl = _ceil_div(cap, NT)      # moving-dim slices per expert batch

    def nslice(n):
        lo = n * NT
        return lo, min(NT, cap - lo)

    with tile.TileContext(nc) as tc, ExitStack() as ctx:
        const = ctx.enter_context(tc.tile_pool(name="resident", bufs=1))
        psum = ctx.enter_context(tc.tile_pool(name="psum", bufs=8, space="PSUM"))
        work = ctx.enter_context(tc.tile_pool(name="work", bufs=3))
        wpool = ctx.enter_context(tc.tile_pool(name="wstream", bufs=6))
        epool = ctx.enter_context(tc.tile_pool(name="eact", bufs=2))

        # ---- resident loads -------------------------------------------------
        # Everything loads through the sync queue so FIFO order holds:
        # small activation inputs FIRST, then the bulk weight streams (the
        # 16 DMA engines drain descriptors in issue order). Stores go on
        # gpsimd.
        xT_sb = const.tile([P, KH, TS], BF)
        sg_sb = const.tile([P, KH, FS], BF)
        su_sb = const.tile([P, KH, FS], BF)
        sd_sb = const.tile([P, FS // P, H], BF)
        xe_sb = [const.tile([P, KH, cap], BF, tag=f"xe_sb{s}", name=f"xe_sb{s}")
                 for s in range(EPC)]
        wt_sb = [const.tile([P, ncp, 1], F32, tag=f"wt_sb{s}", name=f"wt_sb{s}")
                 for s in range(EPC)]
        nc.sync.dma_start(xT_sb[:, 0:4, :], xT[:, 0:4, :])
        nc.sync.dma_start(sg_sb[:, 0:8, :], sg[:, 0:8, :])
        nc.sync.dma_start(su_sb[:, 0:8, :], su[:, 0:8, :])
        nc.sync.dma_start(xe_sb[0][:], xe[0][:])
        nc.sync.dma_start(xT_sb[:, 4:8, :], xT[:, 4:8, :])
        nc.sync.dma_start(sg_sb[:, 8:16, :], sg[:, 8:16, :])
        nc.sync.dma_start(su_sb[:, 8:16, :], su[:, 8:16, :])
        nc.sync.dma_start(xT_sb[:, 8:16, :], xT[:, 8:16, :])
        nc.sync.dma_start(xe_sb[1][:], xe[1][:])
        nc.sync.dma_start(sd_sb[:], sd[:])
        for s in range(EPC):
            for c in range(ncp):
                mc = min(P, cap - c * P)
                nc.sync.dma_start(wt_sb[s][:mc, c, :],
                                  wt[s][c * P:c * P + mc, :])

        # ---- shared expert gate/up (TP shard on F) --------------------------
        # act_s[FS, T] = silu(sg_shard.T @ x.T) * (su_shard.T @ x.T)
        # HAM warmup: ~4us of junk matmuls during the DMA ramp so real
        # matmuls start at 2.4 GHz. Results land in the first psum tile
        # and are discarded by the real chain's start=True.
        warm = const.tile([P, P], BF)
        nc.gpsimd.memset(warm[:], 0.0)
        pwarm = psum.tile([P, NT], F32, tag="mm", name="pwarm")
        for i in range(40):
            nc.tensor.matmul(pwarm[:, :P], warm[:], warm[:],
                             start=True, stop=True)

        # act_s[FS, TS] as 2 f-tiles of [128, TS]
        act_s = const.tile([P, FS // P, TS], BF)
        for f2 in range(FS // P):
            pg = psum.tile([P, NT], F32, tag="mm", name=f"pg{f2}")
            pu = psum.tile([P, NT], F32, tag="mm", name=f"pu{f2}")
            for k in range(KH):
                nc.tensor.matmul(pg[:], sg_sb[:, k, f2 * P:(f2 + 1) * P],
                                 xT_sb[:, k, :],
                                 start=(k == 0), stop=(k == KH - 1))
            for k in range(KH):
                nc.tensor.matmul(pu[:], su_sb[:, k, f2 * P:(f2 + 1) * P],
                                 xT_sb[:, k, :],
                                 start=(k == 0), stop=(k == KH - 1))
            tmp = work.tile([P, NT], F32, tag="silu_s", name=f"tmp{f2}")
            nc.scalar.activation(tmp[:], pg[:],
                                 mybir.ActivationFunctionType.Silu)
            nc.vector.tensor_mul(act_s[:, f2, :], tmp[:], pu[:])

        def shared_down(t0, t1):
            # partial down-proj: shared_part[TS, H] = act_s.T @ sd_shard
            for t in range(t0, t1):
                ob = work.tile([P, H], BF, tag="out_s", name=f"ob{t}")
                for h in range(H // NT):
                    po = psum.tile([P, NT], F32, tag="mm", name=f"po{t}_{h}")
                    for f2 in range(FS // P):
                        nc.tensor.matmul(po[:],
                                         act_s[:, f2, t * P:(t + 1) * P],
                                         sd_sb[:, f2, h * NT:(h + 1) * NT],
                                         start=(f2 == 0),
                                         stop=(f2 == FS // P - 1))
                    # alternate DVE/ACT so neither FIFO gates the gu/down
                    # boundary evictions
                    if (t * 4 + h) % 2 == 0:
                        nc.vector.tensor_copy(ob[:, h * NT:(h + 1) * NT], po[:])
                    else:
                        nc.scalar.activation(ob[:, h * NT:(h + 1) * NT], po[:],
                                             mybir.ActivationFunctionType.Copy)
                nc.gpsimd.dma_start(shp[t * P:(t + 1) * P, :], ob[:])

        def expert_gu_half(s, half, silu_g, act_e):
            # one half of guT[2F, cap] over H: 8 accumulation groups, one
            # PSUM bank per group (packing groups into a bank is a HW fault)
            wsrc = wg[s] if half == 0 else wu[s]
            assert cap <= NT, "expert batch over 512 tokens unsupported"
            ps = [psum.tile([P, cap], F32, tag="mm", name=f"ps_{s}_{half}_{f}")
                  for f in range(KF)]
            for k0 in range(0, KH, 4):  # 1MB 4-k-tile chunks
                wchunk = wpool.tile([P, 4, F], BF, tag="wgu_chunk",
                                    name=f"wgu_{s}_{half}_{k0}")
                nc.sync.dma_start(wchunk[:], wsrc[:, k0:k0 + 4, :])
                for dk in range(4):
                    k = k0 + dk
                    for f in range(KF):
                        nc.tensor.matmul(
                            ps[f][:],
                            wchunk[:, dk, f * P:(f + 1) * P],
                            xe_sb[s][:, k, :],
                            start=(k == 0), stop=(k == KH - 1))
            for f in range(KF):
                if half == 0:  # gate -> silu
                    nc.scalar.activation(silu_g[:, f, :], ps[f][:],
                                         mybir.ActivationFunctionType.Silu)
                else:          # up -> silu(g) * u
                    nc.vector.tensor_mul(act_e[:, f, :],
                                         silu_g[:, f, :], ps[f][:])

        def expert_down(s, act_e):
            # down-proj: y[cap, H] = (act_e.T @ wd) * wt, (c, h) psum groups
            pairs = [(c, h) for c in range(ncp) for h in range(H // NT)]
            for g0 in range(0, len(pairs), 8):
                grp = pairs[g0:g0 + 8]
                pY = {ch: psum.tile([P, NT], F32, tag="mm",
                                    name=f"pY_{s}_{ch[0]}_{ch[1]}")
                      for ch in grp}
                for fk0 in range(0, KF, 2):  # 2 f-strips per DMA (1MB)
                    wdstrip = wpool.tile([P, 2, H], BF, tag="wd_strip",
                                         name=f"wd_{s}_{fk0}")
                    nc.sync.dma_start(wdstrip[:], wd[s][:, fk0:fk0 + 2, :])
                    for dk in range(2):
                        fk = fk0 + dk
                        for (c, h) in grp:
                            mc = min(P, cap - c * P)
                            nc.tensor.matmul(
                                pY[(c, h)][:mc, :],
                                act_e[:, fk, c * P:c * P + mc],
                                wdstrip[:, dk, h * NT:(h + 1) * NT],
                                start=(fk == 0), stop=(fk == KF - 1))
                ycs = {}
                for (c, h) in grp:
                    mc = min(P, cap - c * P)
                    if c not in ycs:
                        ycs[c] = work.tile([P, H], BF, tag="yout",
                                           name=f"yb_{s}_{c}_{h}")
                    dst = ycs[c][:mc, h * NT:(h + 1) * NT]
                    if h % 2 == 0:  # split evictions across DVE and ACT
                        nc.vector.tensor_scalar_mul(dst, pY[(c, h)][:mc, :],
                                                    wt_sb[s][:mc, c, :])
                    else:
                        nc.scalar.activation(dst, pY[(c, h)][:mc, :],
                                             mybir.ActivationFunctionType.Copy,
                                             scale=wt_sb[s][:mc, c, :])
                for c, yb in ycs.items():
                    mc = min(P, cap - c * P)
                    nc.gpsimd.dma_start(ys[s][c * P:c * P + mc, :], yb[:mc, :])

        # order: shared-down after down(0), where its serialized
        # MM->evict chain hides under the DMA-paced expert weight streams
        eact = [(epool.tile([P, KF, cap], F32, tag="silu_g", name=f"silu_g{s}"),
                 epool.tile([P, KF, cap], BF, tag="act_e", name=f"act_e{s}"))
                for s in range(EPC)]
        shared_down(0, TS // P)
        expert_gu_half(0, 0, *eact[0])
        expert_gu_half(0, 1, *eact[0])
        expert_down(0, eact[0][1])
        expert_gu_half(1, 0, *eact[1])
        expert_gu_half(1, 1, *eact[1])
        expert_down(1, eact[1][1])

    nc.compile()
    return nc


def _route_host(x, gate_w, expert_bias):
    """Replicates the reference router in fp32 numpy."""
    logits = x @ gate_w                                    # [T, E]
    scores = 1.0 / (1.0 + np.exp(-logits, dtype=np.float32))
    sel = np.argsort(-(scores + expert_bias[None, :]), axis=1, kind="stable")[:, :TOPK]
    top = np.take_along_axis(scores, sel, axis=1)          # [T, K]
    top = top / (top.sum(-1, keepdims=True) + 1e-20)
    top = top * ROUTE_SCALE
    return sel, top.astype(np.float32)


def _ensure_ntff_hook():
    """The image's antenv lacks axon_hooks; inject it and register the
    NTFF profile hook so trace=True yields exec_time_ns."""
    import types
    try:
        from antenv import axon_hooks  # noqa: F401
        return
    except ImportError:
        pass
    try:
        import antenv
        from trn_agent_boot.trn_boot import _ntff_profile_via_ctypes
        mod = types.ModuleType("antenv.axon_hooks")
        mod._hook = None

        def _set(h):
            mod._hook = h

        def _get():
            return mod._hook

        mod.set_axon_ntff_profile_hook = _set
        mod.get_axon_ntff_profile_hook = _get
        sys.modules["antenv.axon_hooks"] = mod
        antenv.axon_hooks = mod
        _set(_ntff_profile_via_ctypes("/opt/axon/libaxon_pjrt.so"))
    except Exception as e:  # tracing degrades, run still works
        print(f"ntff hook setup failed: {e}")


def _run(inputs, trace=False, trace_cores=None):
    if trace:
        _ensure_ntff_hook()
    x = np.asarray(inputs["hidden_states"], np.float32).reshape(T, H)
    gate_w = np.asarray(inputs["gate_w"], np.float32)
    expert_bias = np.asarray(inputs["expert_bias"], np.float32)
    sgw = np.asarray(inputs["shared_gate_w"], np.float32)
    suw = np.asarray(inputs["shared_up_w"], np.float32)
    sdw = np.asarray(inputs["shared_down_w"], np.float32)
    egu = np.asarray(inputs["experts_gate_up"], np.float32)
    edn = np.asarray(inputs["experts_down"], np.float32)

    # --- host router + dispatch (the shard step) ---
    sel, top = _route_host(x, gate_w, expert_bias)
    idx = [np.where(sel == e)[0] for e in range(E)]        # token ids per expert
    wts = [top[sel == e] for e in range(E)]                # routing weight per token
    cap = max(8, -(-max(len(i) for i in idx) // 16) * 16)  # pad to mult of 16
    if cap > T:
        cap = T

    def ptile(a):
        """[K*128, f] row-major -> [128, K, f] partition-major bf16."""
        k = a.shape[0] // P
        return np.ascontiguousarray(
            a.reshape(k, P, a.shape[1]).transpose(1, 0, 2)).astype(bf16)

    def gathered(e):
        xt = np.zeros((H, cap), np.float32)
        n = len(idx[e])
        xt[:, :n] = x[idx[e]].T
        w = np.zeros((cap, 1), np.float32)
        w[:n, 0] = wts[e]
        return ptile(xt), w

    # shared expert: 4-way F shard x 2-way token shard
    xT_bf = [ptile(x[th * TS:(th + 1) * TS].T) for th in range(NTS)]
    sg_bf = [ptile(sgw[:, fs * FS:(fs + 1) * FS]) for fs in range(NFS)]
    su_bf = [ptile(suw[:, fs * FS:(fs + 1) * FS]) for fs in range(NFS)]
    sd_bf = [ptile(sdw[fs * FS:(fs + 1) * FS, :]) for fs in range(NFS)]

    in_maps = []
    for core in range(NCORES):
        fs, th = core % NFS, core // NFS
        m = {
            "xT": xT_bf[th],
            "sg": sg_bf[fs],
            "su": su_bf[fs],
            "sd": sd_bf[fs],
        }
        for s in range(EPC):
            e = core * EPC + s
            xt, w = gathered(e)
            m[f"xe{s}"] = xt
            m[f"wt{s}"] = w
            m[f"wg{s}"] = ptile(egu[e][:, :F])
            m[f"wu{s}"] = ptile(egu[e][:, F:])
            m[f"wd{s}"] = ptile(edn[e])
        in_maps.append(m)

    nc = _build_bass(cap)
    res = run_bass_kernel_spmd(nc, in_maps, list(range(NCORES)),
                               trace=trace, trace_cores=trace_cores)

    # --- host combine (unshard) ---
    out = np.zeros((T, H), np.float32)
    for core in range(NCORES):
        th = core // NFS
        out[th * TS:(th + 1) * TS] += \
            res.results[core]["shared_part"].astype(np.float32)
        for s in range(EPC):
            e = core * EPC + s
            n = len(idx[e])
            if n:  # token ids within one expert are unique -> plain fancy add
                out[idx[e]] += res.results[core][f"y{s}"][:n].astype(np.float32)
    return out.reshape(B, S, H), res


def kernel(**inputs) -> np.ndarray:
    out, _ = _run(inputs)
    return out


# revision 54
# speedup vs baseline: 1.1805x; 1.1052x over previous
"""AfmoeSparseMoeBlock on 8 Trainium2 NeuronCores.

Strategy (expert-parallel, per sharding hint):
  - Router (tiny: [T,H]@[H,16] + sigmoid + top-2) runs on host as part of
    the dispatch/shard step; it determines which token rows are shipped to
    which core.
  - Experts are sharded 2-per-core. Each core receives its two experts'
    weights plus the (transposed, bf16) gathered token batches, computes
    gate_up -> SwiGLU -> down, and scales rows by the routing weights.
  - The shared-expert MLP is tensor-parallel on its intermediate dim F:
    each core computes a 128-wide slice of the SwiGLU intermediate and a
    partial down-projection; host sums the 8 partials.
  - Combine: host scatter-adds the (already weighted) expert outputs.

Device compute is bf16 (fp32 accumulation in PSUM); outputs are fp32.
"""

import sys
from contextlib import ExitStack

sys.path.insert(0, "/opt/trn_rl_repo")

import numpy as np
import ml_dtypes

import concourse.bass as bass
import concourse.tile as tile
from concourse import bacc, mybir
from concourse.bass_utils import run_bass_kernel_spmd

# Problem constants (nn_AfmoeSparseMoeBlock_38422777430201)
B, S, H = 1, 1024, 2048
F = 1024          # moe_intermediate_size
E = 16            # num_experts
TOPK = 2
ROUTE_SCALE = 1.0
NCORES = 8
EPC = E // NCORES           # experts per core = 2
NFS = 4                     # shared expert: 4-way F shard x 2-way token shard
NTS = NCORES // NFS
FS = F // NFS               # shared-expert intermediate shard = 256
TS = (B * S) // NTS         # shared-expert token shard = 512
T = B * S

P = 128
KH = H // P      # 16 k-tiles over H
KF = F // P      # 8 k-tiles over F
NT = 512         # matmul moving free-dim / PSUM bank width (fp32)

BF = mybir.dt.bfloat16
F32 = mybir.dt.float32
bf16 = ml_dtypes.bfloat16


def _ceil_div(a, b):
    return (a + b - 1) // b


def _build_bass(caps) -> bass.Bass:
    """One SPMD program: shared-expert TP shard + 2 routed experts.

    All DRAM inputs are host-pre-tiled to [128, k, f] (partition-major) so
    each DMA descriptor covers a long contiguous per-partition segment --
    descriptor generation on the DGE sequencers is the scarce resource.
    """
    nc = bacc.Bacc("TRN2", target_bir_lowering=False, debug=False,
                   num_devices=NCORES)

    xT = nc.declare_dram_parameter("xT", [P, KH, TS], BF, isOutput=False)
    sg = nc.declare_dram_parameter("sg", [P, KH, FS], BF, isOutput=False)
    su = nc.declare_dram_parameter("su", [P, KH, FS], BF, isOutput=False)
    sd = nc.declare_dram_parameter("sd", [P, FS // P, H], BF, isOutput=False)
    xe = [nc.declare_dram_parameter(f"xe{s}", [P, KH, caps[s]], BF,
                                    isOutput=False) for s in range(EPC)]
    wg = [nc.declare_dram_parameter(f"wg{s}", [P, KH, F], BF, isOutput=False)
          for s in range(EPC)]
    wu = [nc.declare_dram_parameter(f"wu{s}", [P, KH, F], BF, isOutput=False)
          for s in range(EPC)]
    wd = [nc.declare_dram_parameter(f"wd{s}", [P, KF, H], BF, isOutput=False)
          for s in range(EPC)]
    wt = [nc.declare_dram_parameter(f"wt{s}", [caps[s], 1], F32,
                                    isOutput=False) for s in range(EPC)]
    shp = nc.declare_dram_parameter("shared_part", [TS, H], BF, isOutput=True)
    ys = [nc.declare_dram_parameter(f"y{s}", [caps[s], H], BF, isOutput=True)
          for s in range(EPC)]

    ncps = [_ceil_div(c, P) for c in caps]  # token tiles per expert batch

    with tile.TileContext(nc) as tc, ExitStack() as ctx:
        const = ctx.enter_context(tc.tile_pool(name="resident", bufs=1))
        psum = ctx.enter_context(tc.tile_pool(name="psum", bufs=8, space="PSUM"))
        work = ctx.enter_context(tc.tile_pool(name="work", bufs=3))
        wpool = ctx.enter_context(tc.tile_pool(name="wstream", bufs=6))
        epool = ctx.enter_context(tc.tile_pool(name="eact", bufs=2))

        # ---- resident loads -------------------------------------------------
        # Everything loads through the sync queue so FIFO order holds:
        # small activation inputs FIRST, then the bulk weight streams (the
        # 16 DMA engines drain descriptors in issue order). Stores go on
        # gpsimd.
        xT_sb = const.tile([P, KH, TS], BF)
        sg_sb = const.tile([P, KH, FS], BF)
        su_sb = const.tile([P, KH, FS], BF)
        sd_sb = const.tile([P, FS // P, H], BF)
        xe_sb = [const.tile([P, KH, caps[s]], BF, tag=f"xe_sb{s}",
                         name=f"xe_sb{s}") for s in range(EPC)]
        wt_sb = [const.tile([P, ncps[s], 1], F32, tag=f"wt_sb{s}",
                 name=f"wt_sb{s}") for s in range(EPC)]
        nc.sync.dma_start(xT_sb[:, 0:4, :], xT[:, 0:4, :])
        nc.sync.dma_start(sg_sb[:, 0:8, :], sg[:, 0:8, :])
        nc.sync.dma_start(su_sb[:, 0:8, :], su[:, 0:8, :])
        nc.sync.dma_start(xe_sb[0][:], xe[0][:])
        nc.sync.dma_start(xT_sb[:, 4:8, :], xT[:, 4:8, :])
        nc.sync.dma_start(sg_sb[:, 8:16, :], sg[:, 8:16, :])
        nc.sync.dma_start(su_sb[:, 8:16, :], su[:, 8:16, :])
        nc.sync.dma_start(xT_sb[:, 8:16, :], xT[:, 8:16, :])
        nc.sync.dma_start(xe_sb[1][:], xe[1][:])
        nc.sync.dma_start(sd_sb[:], sd[:])
        for s in range(EPC):
            for c in range(ncps[s]):
                mc = min(P, caps[s] - c * P)
                nc.sync.dma_start(wt_sb[s][:mc, c, :],
                                  wt[s][c * P:c * P + mc, :])

        # ---- shared expert gate/up (TP shard on F) --------------------------
        # act_s[FS, T] = silu(sg_shard.T @ x.T) * (su_shard.T @ x.T)
        # HAM warmup: ~4us of junk matmuls during the DMA ramp so real
        # matmuls start at 2.4 GHz. Results land in the first psum tile
        # and are discarded by the real chain's start=True.
        warm = const.tile([P, P], BF)
        nc.gpsimd.memset(warm[:], 0.0)
        pwarm = psum.tile([P, NT], F32, tag="mm", name="pwarm")
        for i in range(40):
            nc.tensor.matmul(pwarm[:, :P], warm[:], warm[:],
                             start=True, stop=True)

        # act_s[FS, TS] as 2 f-tiles of [128, TS]
        act_s = const.tile([P, FS // P, TS], BF)
        for f2 in range(FS // P):
            pg = psum.tile([P, NT], F32, tag="mm", name=f"pg{f2}")
            pu = psum.tile([P, NT], F32, tag="mm", name=f"pu{f2}")
            for k in range(KH):
                nc.tensor.matmul(pg[:], sg_sb[:, k, f2 * P:(f2 + 1) * P],
                                 xT_sb[:, k, :],
                                 start=(k == 0), stop=(k == KH - 1))
            for k in range(KH):
                nc.tensor.matmul(pu[:], su_sb[:, k, f2 * P:(f2 + 1) * P],
                                 xT_sb[:, k, :],
                                 start=(k == 0), stop=(k == KH - 1))
            tmp = work.tile([P, NT], F32, tag="silu_s", name=f"tmp{f2}")
            nc.scalar.activation(tmp[:], pg[:],
                                 mybir.ActivationFunctionType.Silu)
            nc.vector.tensor_mul(act_s[:, f2, :], tmp[:], pu[:])

        def shared_down(t0, t1):
            # partial down-proj: shared_part[TS, H] = act_s.T @ sd_shard
            for t in range(t0, t1):
                ob = work.tile([P, H], BF, tag="out_s", name=f"ob{t}")
                for h in range(H // NT):
                    po = psum.tile([P, NT], F32, tag="mm", name=f"po{t}_{h}")
                    for f2 in range(FS // P):
                        nc.tensor.matmul(po[:],
                                         act_s[:, f2, t * P:(t + 1) * P],
                                         sd_sb[:, f2, h * NT:(h + 1) * NT],
                                         start=(f2 == 0),
                                         stop=(f2 == FS // P - 1))
                    # alternate DVE/ACT so neither FIFO gates the gu/down
                    # boundary evictions
                    if (t * 4 + h) % 2 == 0:
                        nc.vector.tensor_copy(ob[:, h * NT:(h + 1) * NT], po[:])
                    else:
                        nc.scalar.activation(ob[:, h * NT:(h + 1) * NT], po[:],
                                             mybir.ActivationFunctionType.Copy)
                nc.gpsimd.dma_start(shp[t * P:(t + 1) * P, :], ob[:])

        def expert_gu_half(s, half, silu_g, act_e):
            # one half of guT[2F, cap] over H: 8 accumulation groups, one
            # PSUM bank per group (packing groups into a bank is a HW fault)
            wsrc = wg[s] if half == 0 else wu[s]
            cap = caps[s]
            assert cap <= NT, "expert batch over 512 tokens unsupported"
            ps = [psum.tile([P, cap], F32, tag="mm", name=f"ps_{s}_{half}_{f}")
                  for f in range(KF)]
            for k0 in range(0, KH, 4):  # 1MB 4-k-tile chunks
                wchunk = wpool.tile([P, 4, F], BF, tag="wgu_chunk",
                                    name=f"wgu_{s}_{half}_{k0}")
                nc.sync.dma_start(wchunk[:], wsrc[:, k0:k0 + 4, :])
                for dk in range(4):
                    k = k0 + dk
                    for f in range(KF):
                        nc.tensor.matmul(
                            ps[f][:],
                            wchunk[:, dk, f * P:(f + 1) * P],
                            xe_sb[s][:, k, :],
                            start=(k == 0), stop=(k == KH - 1))
            for f in range(KF):
                if half == 0:  # gate -> silu
                    nc.scalar.activation(silu_g[:, f, :], ps[f][:],
                                         mybir.ActivationFunctionType.Silu)
                else:          # up -> silu(g) * u
                    nc.vector.tensor_mul(act_e[:, f, :],
                                         silu_g[:, f, :], ps[f][:])

        def expert_down(s, act_e):
            # down-proj: y[cap, H] = (act_e.T @ wd) * wt, (c, h) psum groups
            cap = caps[s]
            pairs = [(c, h) for c in range(_ceil_div(cap, P))
                 for h in range(H // NT)]
            for g0 in range(0, len(pairs), 8):
                grp = pairs[g0:g0 + 8]
                pY = {ch: psum.tile([P, NT], F32, tag="mm",
                                    name=f"pY_{s}_{ch[0]}_{ch[1]}")
                      for ch in grp}
                for fk0 in range(0, KF, 2):  # 2 f-strips per DMA (1MB)
                    wdstrip = wpool.tile([P, 2, H], BF, tag="wd_strip",
                                         name=f"wd_{s}_{fk0}")
                    nc.sync.dma_start(wdstrip[:], wd[s][:, fk0:fk0 + 2, :])
                    for dk in range(2):
                        fk = fk0 + dk
                        for (c, h) in grp:
                            mc = min(P, cap - c * P)
                            nc.tensor.matmul(
                                pY[(c, h)][:mc, :],
                                act_e[:, fk, c * P:c * P + mc],
                                wdstrip[:, dk, h * NT:(h + 1) * NT],
                                start=(fk == 0), stop=(fk == KF - 1))
                ycs = {}
                for (c, h) in grp:
                    mc = min(P, cap - c * P)
                    if c not in ycs:
                        ycs[c] = work.tile([P, H], BF, tag="yout",
                                           name=f"yb_{s}_{c}_{h}")
                    dst = ycs[c][:mc, h * NT:(h + 1) * NT]
                    if h % 2 == 0:  # split evictions across DVE and ACT
                        nc.vector.tensor_scalar_mul(dst, pY[(c, h)][:mc, :],
                                                    wt_sb[s][:mc, c, :])
                    else:
                        nc.scalar.activation(dst, pY[(c, h)][:mc, :],
                                             mybir.ActivationFunctionType.Copy,
                                             scale=wt_sb[s][:mc, c, :])
                for c, yb in ycs.items():
                    mc = min(P, cap - c * P)
                    nc.gpsimd.dma_start(ys[s][c * P:c * P + mc, :], yb[:mc, :])

        # order: shared-down after down(0), where its serialized
        # MM->evict chain hides under the DMA-paced expert weight streams
        eact = [(epool.tile([P, KF, caps[s]], F32, tag="silu_g",
                             name=f"silu_g{s}"),
                 epool.tile([P, KF, caps[s]], BF, tag="act_e",
                            name=f"act_e{s}"))
                for s in range(EPC)]
        shared_down(0, TS // P)
        expert_gu_half(0, 0, *eact[0])
        expert_gu_half(0, 1, *eact[0])
        expert_down(0, eact[0][1])
        expert_gu_half(1, 0, *eact[1])
        expert_gu_half(1, 1, *eact[1])
        expert_down(1, eact[1][1])

    nc.compile()
    return nc


def _route_host(x, gate_w, expert_bias):
    """Replicates the reference router in fp32 numpy."""
    logits = x @ gate_w                                    # [T, E]
    scores = 1.0 / (1.0 + np.exp(-logits, dtype=np.float32))
    sel = np.argsort(-(scores + expert_bias[None, :]), axis=1, kind="stable")[:, :TOPK]
    top = np.take_along_axis(scores, sel, axis=1)          # [T, K]
    top = top / (top.sum(-1, keepdims=True) + 1e-20)
    top = top * ROUTE_SCALE
    return sel, top.astype(np.float32)


def _ensure_ntff_hook():
    """The image's antenv lacks axon_hooks; inject it and register the
    NTFF profile hook so trace=True yields exec_time_ns."""
    import types
    try:
        from antenv import axon_hooks  # noqa: F401
        return
    except ImportError:
        pass
    try:
        import antenv
        from trn_agent_boot.trn_boot import _ntff_profile_via_ctypes
        mod = types.ModuleType("antenv.axon_hooks")
        mod._hook = None

        def _set(h):
            mod._hook = h

        def _get():
            return mod._hook

        mod.set_axon_ntff_profile_hook = _set
        mod.get_axon_ntff_profile_hook = _get
        sys.modules["antenv.axon_hooks"] = mod
        antenv.axon_hooks = mod
        _set(_ntff_profile_via_ctypes("/opt/axon/libaxon_pjrt.so"))
    except Exception as e:  # tracing degrades, run still works
        print(f"ntff hook setup failed: {e}")


def _run(inputs, trace=False, trace_cores=None):
    if trace:
        _ensure_ntff_hook()
    x = np.asarray(inputs["hidden_states"], np.float32).reshape(T, H)
    gate_w = np.asarray(inputs["gate_w"], np.float32)
    expert_bias = np.asarray(inputs["expert_bias"], np.float32)
    sgw = np.asarray(inputs["shared_gate_w"], np.float32)
    suw = np.asarray(inputs["shared_up_w"], np.float32)
    sdw = np.asarray(inputs["shared_down_w"], np.float32)
    egu = np.asarray(inputs["experts_gate_up"], np.float32)
    edn = np.asarray(inputs["experts_down"], np.float32)

    # --- host router + dispatch (the shard step) ---
    sel, top = _route_host(x, gate_w, expert_bias)
    idx = [np.where(sel == e)[0] for e in range(E)]        # token ids per expert
    wts = [top[sel == e] for e in range(E)]                # routing weight per token

    # slot 0 takes the 8 heaviest experts, slot 1 the rest, so each slot's
    # padded capacity is minimal (caps are compile-time constants)
    order = sorted(range(E), key=lambda e: -len(idx[e]))
    emap = [[0] * EPC for _ in range(NCORES)]  # core, slot -> expert id
    for r, e in enumerate(order):
        emap[r % NCORES][r // NCORES] = e

    def roundcap(n):
        return min(T, max(8, -(-n // 16) * 16))  # pad to mult of 16

    caps = tuple(roundcap(max(len(idx[emap[c][s]]) for c in range(NCORES)))
                 for s in range(EPC))

    def ptile(a):
        """[K*128, f] row-major -> [128, K, f] partition-major bf16."""
        k = a.shape[0] // P
        return np.ascontiguousarray(
            a.reshape(k, P, a.shape[1]).transpose(1, 0, 2)).astype(bf16)

    def gathered(e, cap):
        xt = np.zeros((H, cap), np.float32)
        n = len(idx[e])
        xt[:, :n] = x[idx[e]].T
        w = np.zeros((cap, 1), np.float32)
        w[:n, 0] = wts[e]
        return ptile(xt), w

    # shared expert: 4-way F shard x 2-way token shard
    xT_bf = [ptile(x[th * TS:(th + 1) * TS].T) for th in range(NTS)]
    sg_bf = [ptile(sgw[:, fs * FS:(fs + 1) * FS]) for fs in range(NFS)]
    su_bf = [ptile(suw[:, fs * FS:(fs + 1) * FS]) for fs in range(NFS)]
    sd_bf = [ptile(sdw[fs * FS:(fs + 1) * FS, :]) for fs in range(NFS)]

    in_maps = []
    for core in range(NCORES):
        fs, th = core % NFS, core // NFS
        m = {
            "xT": xT_bf[th],
            "sg": sg_bf[fs],
            "su": su_bf[fs],
            "sd": sd_bf[fs],
        }
        for s in range(EPC):
            e = emap[core][s]
            xt, w = gathered(e, caps[s])
            m[f"xe{s}"] = xt
            m[f"wt{s}"] = w
            m[f"wg{s}"] = ptile(egu[e][:, :F])
            m[f"wu{s}"] = ptile(egu[e][:, F:])
            m[f"wd{s}"] = ptile(edn[e])
        in_maps.append(m)

    nc = _build_bass(caps)
    res = run_bass_kernel_spmd(nc, in_maps, list(range(NCORES)),
                               trace=trace, trace_cores=trace_cores)

    # --- host combine (unshard) ---
    out = np.zeros((T, H), np.float32)
    for core in range(NCORES):
        th = core // NFS
        out[th * TS:(th + 1) * TS] += \
            res.results[core]["shared_part"].astype(np.float32)
        for s in range(EPC):
            e = emap[core][s]
            n = len(idx[e])
            if n:  # token ids within one expert are unique -> plain fancy add
                out[idx[e]] += res.results[core][f"y{s}"][:n].astype(np.float32)
    return out.reshape(B, S, H), res


def kernel(**inputs) -> np.ndarray:
    out, _ = _run(inputs)
    return out
